# revision 5
# baseline (speedup 1.0000x reference)
"""Trainium2 Bass kernel for nn_CleanupBlock (chain of 4 CleanUpKV layers).

Math (reference): per layer l: hn = h/||h||; s = hn @ K_l^T; w = softmax(s);
h = w @ V_l.

Folded algorithm used here (exact in real arithmetic, verified ~1e-5 rel in
fp32): softmax numerators e_l are carried unnormalized; since h_l = w_l @ V_l
and w = e/sum(e), consecutive layers fold into [256,256] operators:
    scores_l = e_{l-1} @ M_l / sqrt(e G e^T),  M_l = V_{l-1} K_l^T,
    G_{l-1} = V_{l-1} V_{l-1}^T   (the sum(e) temperature cancels, and
    ||h||^2 = (e G e)/sum(e)^2 supplies the normalization).
Final: out = (e_4 / sum(e_4)) @ V_4.

On-chip layout is fully "transposed": activations live as [feature, token]
tiles so the support dim (256) is the matmul contraction on partitions, the
fixed [256,512] operators are stationary, and per-token reductions
(norms / softmax denominators) are ones-vector matmuls that also broadcast
the result across all 128 partitions for free.

Sharding: pure data-parallel over the 16384 tokens across 8 cores (2048
tokens/core); the small operator tensors are replicated.
"""

import numpy as np

import concourse.bacc as bacc
import concourse.tile as tile
from concourse import mybir
from concourse.bass import ts
from concourse.bass_utils import run_bass_kernel_spmd

F32 = mybir.dt.float32

# matmul operand dtype: float32r (full-rate reduced-precision fp32) by default
MM_DT = mybir.dt.float32r

B, S, D = 4, 4096, 1024
NS = 256          # num_support
NCORES = 8
T = (B * S) // NCORES   # tokens per core = 2048
TC = 512                # token chunk (matmul free dim / PSUM bank)
NCH = T // TC           # chunks per core = 4
DC = D // 128           # d-chunks = 8
NM = NS // 128          # support chunks = 2

_CACHE = {}


def _patch_act_tables():
    """Force Exp/Ln/Square onto the combined natural_log_exp_and_others ACT
    table set. The default chooser picks exp_and_others for Exp/Square and
    natural_log for Ln, inserting a ~2.7us ACT_TABLE_LOAD swap before almost
    every activation (33 loads/kernel); with all three pinned to the one set
    that contains them all, a single load serves the whole kernel."""
    import concourse.bacc as _bacc
    if getattr(_bacc, "_act_tables_patched", False):
        return
    _orig = _bacc.get_activation_tables
    _special = {
        mybir.ActivationFunctionType.Exp,
        mybir.ActivationFunctionType.Ln,
        mybir.ActivationFunctionType.Square,
    }

    def _patched(module_arch):
        tabs = _orig(module_arch)
        return {
            name: (funcs if name == "natural_log_exp_and_others"
                   else funcs - _special)
            for name, funcs in tabs.items()
        }

    _bacc.get_activation_tables = _patched
    _bacc._act_tables_patched = True


def _build():
    _patch_act_tables()
    nc = bacc.Bacc("TRN2", target_bir_lowering=False, debug=False,
                   num_devices=NCORES)

    xt_d = nc.dram_tensor("xt", [D, T], F32, kind="ExternalInput")
    k1t_d = nc.dram_tensor("k1t", [D, NS], F32, kind="ExternalInput")
    mg_d = nc.dram_tensor("mg", [3, NS, 512], F32, kind="ExternalInput")
    v4_d = nc.dram_tensor("v4", [NS, D], F32, kind="ExternalInput")
    out_d = nc.dram_tensor("outt", [D, T], F32, kind="ExternalOutput")

    Exp = mybir.ActivationFunctionType.Exp
    Ln = mybir.ActivationFunctionType.Ln
    Square = mybir.ActivationFunctionType.Square

    with tile.TileContext(nc) as tc:
        with (
            tc.tile_pool(name="wp", bufs=1) as wp,
            tc.tile_pool(name="xp", bufs=2) as xp,
            tc.tile_pool(name="x2p", bufs=2) as x2p,
            tc.tile_pool(name="ep", bufs=6) as ep,
            tc.tile_pool(name="prp", bufs=4) as prp,
            tc.tile_pool(name="rsp", bufs=8) as rsp,
            tc.tile_pool(name="op", bufs=12) as op,
            tc.tile_pool(name="ps", bufs=8, space="PSUM") as ps,
        ):
            # ---- fixed operators, cast to MM_DT on load (gpsimd DMA casts)
            k1t = wp.tile([128, DC, NS], MM_DT, tag="k1t")
            nc.gpsimd.dma_start(
                out=k1t, in_=k1t_d.ap().rearrange("(c p) n -> p c n", p=128))
            mg = wp.tile([128, 3, NM, 512], MM_DT, tag="mg")
            nc.gpsimd.dma_start(
                out=mg, in_=mg_d.ap().rearrange("l (m p) n -> p l m n", p=128))
            v4 = wp.tile([128, NM, D], MM_DT, tag="v4")
            nc.gpsimd.dma_start(
                out=v4, in_=v4_d.ap().rearrange("(m p) d -> p m d", p=128))
            ones_f = wp.tile([128, 128], F32, tag="onesf")
            nc.vector.memset(ones_f, 1.0)
            ones = wp.tile([128, 128], MM_DT, tag="ones")
            nc.vector.tensor_copy(ones, ones_f)

            for c in range(NCH):
                tsl = slice(c * TC, (c + 1) * TC)

                # ================= layer 1 (from x) =================
                xt = xp.tile([128, DC, TC], MM_DT, tag="xt")
                nc.gpsimd.dma_start(
                    out=xt,
                    in_=xt_d.ap()[:, tsl].rearrange("(dc p) t -> p dc t", p=128))
                x2 = x2p.tile([128, DC, TC], MM_DT, tag="x2")
                nc.scalar.activation(x2, xt, Square)

                ps_s = [ps.tile([128, TC], F32, tag="ps", name=f"ps_s{i}") for i in range(NM)]
                for nt in range(NM):
                    for dc in range(DC):
                        nc.tensor.matmul(
                            ps_s[nt], k1t[:, dc, ts(nt, 128)], xt[:, dc, :],
                            start=(dc == 0), stop=(dc == DC - 1))
                ps_n2 = ps.tile([128, TC], F32, tag="ps")
                for dc in range(DC):
                    nc.tensor.matmul(ps_n2, ones, x2[:, dc, :],
                                     start=(dc == 0), stop=(dc == DC - 1))

                e = self_norm_softmax(nc, rsp, ep, ps_n2, ps_s, Ln, Exp)

                # ================= layers 2..4 =================
                for li in range(3):
                    ps_sg = [ps.tile([128, TC], F32, tag="ps", name=f"ps_sg{i}")
                             for i in range(2 * NM)]
                    for ct in range(2 * NM):
                        for m in range(NM):
                            nc.tensor.matmul(
                                ps_sg[ct], mg[:, li, m, ts(ct, 128)],
                                e[:, m, :],
                                start=(m == 0), stop=(m == NM - 1))
                    pr = prp.tile([128, NM, TC], MM_DT, tag="pr")
                    for m in range(NM):
                        nc.vector.tensor_mul(pr[:, m, :], ps_sg[NM + m],
                                             e[:, m, :])
                    ps_n2 = ps.tile([128, TC], F32, tag="ps")
                    for m in range(NM):
                        nc.tensor.matmul(ps_n2, ones, pr[:, m, :],
                                         start=(m == 0), stop=(m == NM - 1))
                    e = self_norm_softmax(nc, rsp, ep, ps_n2, ps_sg[:NM],
                                          Ln, Exp)

                # ================= output =================
                ps_d = ps.tile([128, TC], F32, tag="ps")
                for m in range(NM):
                    nc.tensor.matmul(ps_d, ones, e[:, m, :],
                                     start=(m == 0), stop=(m == NM - 1))
                rd = rsp.tile([128, TC], F32, tag="rs")
                nc.vector.reciprocal(rd, ps_d)
                es = prp.tile([128, NM, TC], MM_DT, tag="pr")
                for m in range(NM):
                    nc.vector.tensor_mul(es[:, m, :], e[:, m, :], rd)
                for dt_ in range(DC):
                    ps_o = ps.tile([128, TC], F32, tag="ps")
                    for m in range(NM):
                        nc.tensor.matmul(ps_o, v4[:, m, ts(dt_, 128)],
                                         es[:, m, :],
                                         start=(m == 0), stop=(m == NM - 1))
                    o = op.tile([128, TC], F32, tag="o")
                    if dt_ % 2 == 0:
                        nc.scalar.copy(o, ps_o)
                    else:
                        nc.vector.tensor_copy(o, ps_o)
                    nc.scalar.dma_start(
                        out=out_d.ap()[ts(dt_, 128), tsl], in_=o)

    nc.compile()
    return nc


def self_norm_softmax(nc, rsp, ep, ps_n2, ps_s, Ln, Exp):
    """rs = 1/sqrt(n2) = exp(-0.5*ln(n2)); e = exp(scores * rs).

    ln/exp live in one ACT table set (natural_log_exp_and_others) so the
    whole kernel needs a single ACT_TABLE_LOAD; sqrt would force a ~2.7us
    table swap per layer per chunk."""
    ln2 = rsp.tile([128, TC], F32, tag="rs")
    nc.scalar.activation(ln2, ps_n2, Ln)
    rs = rsp.tile([128, TC], F32, tag="rs")
    nc.scalar.activation(rs, ln2, Exp, scale=-0.5)
    e = ep.tile([128, NM, TC], MM_DT, tag="e")
    for nt in range(NM):
        nc.vector.tensor_mul(ps_s[nt], ps_s[nt], rs)
        nc.scalar.activation(e[:, nt, :], ps_s[nt], Exp)
    return e


def _prep_inputs(x, keys, values):
    xf = np.ascontiguousarray(x.reshape(B * S, D))
    K = keys.astype(np.float64)
    V = values.astype(np.float64)
    k1t = np.ascontiguousarray(keys[0].T).astype(np.float32)
    mg = np.empty([3, NS, 512], np.float32)
    for li, l in enumerate([1, 2, 3]):
        mg[li, :, :NS] = (V[l - 1] @ K[l].T).astype(np.float32)
        mg[li, :, NS:] = (V[l - 1] @ V[l - 1].T).astype(np.float32)
    v4 = np.ascontiguousarray(values[3]).astype(np.float32)
    in_maps = []
    for i in range(NCORES):
        in_maps.append({
            "xt": np.ascontiguousarray(xf[i * T:(i + 1) * T].T),
            "k1t": k1t,
            "mg": mg,
            "v4": v4,
        })
    return in_maps


def kernel(x, keys, values, trace=False):
    x = np.asarray(x, dtype=np.float32)
    keys = np.asarray(keys, dtype=np.float32)
    values = np.asarray(values, dtype=np.float32)
    if "nc" not in _CACHE:
        _CACHE["nc"] = _build()
    nc = _CACHE["nc"]
    in_maps = _prep_inputs(x, keys, values)
    res = run_bass_kernel_spmd(nc, in_maps, core_ids=list(range(NCORES)),
                               trace=trace)
    _CACHE["last_result"] = res
    out = np.concatenate(
        [res.results[i]["outt"].T for i in range(NCORES)], axis=0)
    return np.ascontiguousarray(out.reshape(B, S, D))
